# revision 57
# baseline (speedup 1.0000x reference)
"""Grouped-Query Attention block (RMSNorm + RoPE + causal GQA + o_proj) on 8 trn2 NeuronCores.

Sharding: data-parallel over batch (2) x tensor-parallel over kv-head groups (4).
Core c = b*4 + g handles batch b, kv heads {2g, 2g+1}, q heads {8g..8g+7}.
Each core computes a partial o_proj output (T, D) over its 768 head-dims;
host sums the 4 group partials per batch.

v2 design notes (vs the fp32r baseline):
  * All matmul operands are bf16 (halves DMA + SBUF); accumulation stays fp32
    in PSUM. fp32r and bf16 both stream 1 row/cycle on TRN2, so PE time is
    unchanged but DMA drops ~2x. RMSNorm/softmax scalars stay fp32.
  * Loop order: token-halves INSIDE each projection output, so every weight
    tile is DMA'd exactly once (the old kernel fetched all weights twice).
    x stays fully resident (6.3MB bf16).
  * Attention for head h is software-pipelined INTO the projection stream of
    head h+1: one sc/pv step is emitted every 2 projection matmuls, so the
    exp (Act) chain overlaps PE work instead of gating it.
  * o_proj contraction packed 8x96 -> 6x128 (attention outputs scattered into
    packed tiles via SBUF-SBUF DMA), saving 2 accumulation matmuls per tile.
  * V transposes use the XBAR dma_start_transpose (bf16) instead of PE
    transposes + PSUM round-trip.
  * Engine balance: exp/squares/sqrt/vt/ob on Act, broadcast copies on Pool
    (gpsimd), RoPE (all-bf16, 2x DVE mode) + reciprocals + norm muls on DVE.
  * Causal mask applied structurally: above-diagonal k-tiles skipped,
    diagonal tiles masked with affine_select (fill 0 post-exp).
"""

import os
import sys

import numpy as np

sys.path.insert(0, "/opt/trn_rl_repo")

B, T, D = 2, 1024, 3072
NH, NKV, HD = 32, 8, 96
G = 4                 # tensor-parallel groups
QH = NH // G          # q heads per core (8)
KVH = NKV // G        # kv heads per core (2)
NCORES = 8
EPS = 1e-6
SCALE = 1.0 / float(np.sqrt(HD))
KT = D // 128         # 24 contraction tiles over d_model
TH = 2                # token halves per projection output
THS = T // TH         # 512
QC = 2                # q chunks in attention
QCS = T // QC         # 512
KTOK = T // 128       # 8 k tiles over tokens
NJ = D // 512         # 6 output column chunks
NT = QH * HD // 128   # 6 packed o_proj contraction tiles

_BUILD_CACHE = {}


def _build_nc():
    from contextlib import ExitStack
    from concourse import bacc, tile, mybir

    f32 = mybir.dt.float32
    f32r = mybir.dt.float32r
    bf16 = mybir.dt.bfloat16
    AF = mybir.ActivationFunctionType

    nc = bacc.Bacc("TRN2", target_bir_lowering=False, debug=False,
                   num_devices=NCORES)

    xt_d = nc.dram_tensor("xt", (128, KT, T), bf16, kind="ExternalInput").ap()
    wqt_d = nc.dram_tensor("wqt", (QH, 128, KT, 128), bf16, kind="ExternalInput").ap()
    wkt_d = nc.dram_tensor("wkt", (KVH, 128, KT, 128), bf16, kind="ExternalInput").ap()
    wvt_d = nc.dram_tensor("wvt", (KVH, 128, KT, HD), bf16, kind="ExternalInput").ap()
    wop_d = nc.dram_tensor("wop", (128, NT, D), bf16, kind="ExternalInput").ap()
    # row consts blob: [qnw(128) | knw(128) | ones96(96)] on one partition
    crow_d = nc.dram_tensor("crow", (1, 352), f32r, kind="ExternalInput").ap()
    o128_d = nc.dram_tensor("o128", (128, 1), f32r, kind="ExternalInput").ap()
    # RoPE tables blob: [taba(T) | tabb(T) | ones for vext(8) | swap-perm P]
    tabs_d = nc.dram_tensor("tabs", (128, 2 * T + KTOK + 128), bf16,
                            kind="ExternalInput").ap()
    # sin table in permuted layout (f32: multiplied against PSUM operands)
    tabsw_d = nc.dram_tensor("tabsw", (128, T), f32, kind="ExternalInput").ap()
    out_d = nc.dram_tensor("out", (T, D), f32, kind="ExternalOutput").ap()
    probes = bool(int(os.environ.get("BASS_PROBES", "0")))
    if probes:
        pq_d = nc.dram_tensor("pq", (128, T), bf16, kind="ExternalOutput").ap()
        pk_d = nc.dram_tensor("pk", (128, T), bf16, kind="ExternalOutput").ap()
        pv_d = nc.dram_tensor("pv", (128, KTOK, HD + 1), bf16,
                              kind="ExternalOutput").ap()
        pa_d = nc.dram_tensor("pa", (128, T), bf16, kind="ExternalOutput").ap()

    with tile.TileContext(nc) as tc:
        with nc.allow_low_precision(reason="bf16 matmuls, fp32 accum"), \
             ExitStack() as ctx:
            const = ctx.enter_context(tc.tile_pool(name="const", bufs=1))
            p_per = ctx.enter_context(tc.tile_pool(name="p_per", bufs=1))

            eps_t = const.tile([1, 1], f32, tag="eps")
            nc.vector.memset(eps_t[:], EPS)
            ident = const.tile([128, 128], f32, tag="ident")
            from concourse.masks import make_identity
            make_identity(nc, ident[:])
            crow = const.tile([1, 352], f32r, tag="crow")
            qnw_t = crow[:, 0:128]
            knw_t = crow[:, 128:256]
            ones96 = crow[:, 256:256 + HD]
            ones128 = const.tile([128, 1], f32r, tag="ones128")
            tabs_t = const.tile([128, 2 * T + KTOK + 128], bf16, tag="tabs")
            perm_t = tabs_t[:, 2 * T + KTOK:2 * T + KTOK + 128]
            tabsw_t = const.tile([128, T], f32, tag="tabsw")

            # x resident (both token halves), loaded once. Emission order
            # matters: the serial DMA stream must deliver the first two weight
            # tiles BEFORE x, and x token-half 0 before half 1, so the first
            # projection starts ~1.5us in instead of ~15us.
            xt_t = p_per.tile([128, KT, T], bf16, tag="xt")

            # persistent per-head tensors
            qt = [p_per.tile([128, T], bf16, tag=f"qt{h}", name=f"qt{h}")
                  for h in range(QH)]
            ktl = [p_per.tile([128, T], bf16, tag=f"kt{g2}", name=f"kt{g2}")
                   for g2 in range(KVH)]
            vext = [p_per.tile([128, KTOK, HD + 1], bf16, tag=f"vx{g2}",
                               name=f"vx{g2}") for g2 in range(KVH)]
            atp = [p_per.tile([128, T], bf16, tag=f"atp{t}", name=f"atp{t}")
                   for t in range(NT)]
            wop_t = p_per.tile([128, NT, D], bf16, tag="wop")

            # qt/ktl pad rows get exact zeros from the full-width norm
            # multiply (see part2); vext's softmax-denominator ones column is
            # DMA'd straight from the DRAM tabs blob tail (DRAM->SBUF with a
            # strided destination is the baseline-proven pattern).
            def load_vext_ones():
                for g2 in range(KVH):
                    nc.sync.dma_start(vext[g2][:, :, HD:HD + 1],
                                      tabs_d[:, 2 * T:2 * T + KTOK])

            # attention-side pools (live through phase 3 zip)
            sc_pool = ctx.enter_context(
                tc.tile_pool(name="sc", bufs=2, space="PSUM"))
            po_pool = ctx.enter_context(
                tc.tile_pool(name="po", bufs=2, space="PSUM"))
            mmb_pool = ctx.enter_context(
                tc.tile_pool(name="mmb", bufs=2, space="PSUM"))
            pt_pool = ctx.enter_context(tc.tile_pool(name="pt", bufs=3))
            at_pool = ctx.enter_context(tc.tile_pool(name="att", bufs=2))

            # ---------------- attention step machinery ----------------------
            attn_done = {}  # (head, qc) -> True once normalization emitted

            def make_attn_steps(h):
                g2 = h // (QH // KVH)
                S = [(0, k) for k in range(4)] + [(1, k) for k in range(8)]
                pts = {}
                pos = {}

                def emit_sc(j):
                    qc, kt2 = S[j]
                    qsl = slice(qc * QCS, (qc + 1) * QCS)
                    sc = sc_pool.tile([128, QCS], f32, tag="sc")
                    nc.tensor.matmul(
                        sc[:], ktl[g2][:, kt2 * 128:(kt2 + 1) * 128],
                        qt[h][:, qsl], start=True, stop=True)
                    pt = pt_pool.tile([128, QCS], bf16, tag="pt")
                    nc.scalar.activation(pt[:], sc[:], AF.Exp, scale=SCALE)
                    if kt2 >= qc * (QCS // 128):
                        nc.gpsimd.affine_select(
                            pt[:], pt[:], pattern=[[1, QCS]],
                            compare_op=mybir.AluOpType.is_ge,
                            fill=0.0,
                            base=qc * QCS - kt2 * 128,
                            channel_multiplier=-1)
                    pts[j] = pt

                def emit_fin(qc, po):
                    qsl = slice(qc * QCS, (qc + 1) * QCS)
                    rinv2 = at_pool.tile([1, QCS], f32r, tag="rinv2")
                    nc.vector.reciprocal(rinv2[:], po[HD:HD + 1, :])
                    bc2 = mmb_pool.tile([128, QCS], f32, tag="mmb")
                    nc.tensor.matmul(bc2[0:HD, :], ones96, rinv2[:],
                                     start=True, stop=True)
                    bc2s = at_pool.tile([HD, QCS], f32, tag="bc2s")
                    nc.vector.tensor_copy(bc2s[:], bc2[0:HD, :])
                    att = at_pool.tile([HD, QCS], bf16, tag="attn")
                    nc.vector.tensor_mul(att[:], po[0:HD, :], bc2s[:])
                    s = HD * h
                    t0, o0 = divmod(s, 128)
                    r1 = min(128 - o0, HD)
                    nc.sync.dma_start(atp[t0][o0:o0 + r1, qsl], att[0:r1, :])
                    if r1 < HD:
                        nc.sync.dma_start(atp[t0 + 1][0:HD - r1, qsl],
                                          att[r1:HD, :])
                    attn_done[(h, qc)] = True

                def emit_pv(j, po):
                    qc, kt2 = S[j]
                    last = 3 if qc == 0 else 7
                    nc.tensor.matmul(
                        po[:], vext[g2][:, kt2, :], pts.pop(j)[:],
                        start=(kt2 == 0), stop=(kt2 == last))
                    if kt2 == last:
                        emit_fin(qc, po)

                steps = []
                L = 2
                for j in range(L):
                    steps.append(lambda j=j: emit_sc(j))

                def pv_step(j):
                    qc, kt2 = S[j]
                    if kt2 == 0:
                        pos[qc] = po_pool.tile([HD + 1, QCS], f32, tag="po",
                                               name="po")
                    emit_pv(j, pos[qc])
                    if j + L < len(S):
                        emit_sc(j + L)

                for j in range(len(S)):
                    steps.append(lambda j=j: pv_step(j))
                return steps

            # ---------------- Phase 1+2: projections + zipped attention -----
            steps = []  # pending deferred-emission steps (rms tails + attn)
            with ExitStack() as s1:
                w_pool = s1.enter_context(tc.tile_pool(name="wst", bufs=2))
                t1 = s1.enter_context(tc.tile_pool(name="tmp1", bufs=2))
                ps_pool = s1.enter_context(
                    tc.tile_pool(name="ps1", bufs=2, space="PSUM"))

                def rms_rope(kind, idx, th, ps):
                    """Emit the square inline; defer everything with a PE
                    instruction (ssq/bc matmuls) into the step queue so the
                    Act chain never gates the PE stream."""
                    tsl = slice(th * THS, (th + 1) * THS)
                    dst = qt[idx] if kind == "q" else ktl[idx]
                    nw = qnw_t if kind == "q" else knw_t
                    sq = t1.tile([128, THS], f32r, tag="sq")
                    nc.scalar.square(sq[:], ps[:])
                    box = {}

                    def part1():
                        ssq = mmb_pool.tile([128, THS], f32, tag="mmb",
                                            name="ssq")
                        nc.tensor.matmul(ssq[0:1, :], ones128[:], sq[:],
                                         start=True, stop=True)
                        rms = t1.tile([1, THS], f32, tag="rms")
                        nc.scalar.activation(rms[:], ssq[0:1, :], AF.Sqrt,
                                             bias=eps_t[:], scale=1.0 / HD)
                        rinv = t1.tile([1, THS], f32r, tag="rinv")
                        nc.vector.reciprocal(rinv[:], rms[:])
                        box["rinv"] = rinv

                    def part2():
                        bc = mmb_pool.tile([128, THS], f32, tag="mmb",
                                           name="bc")
                        nc.tensor.matmul(bc[:], nw, box["rinv"][:],
                                         start=True, stop=True)
                        bcs = t1.tile([128, THS], f32, tag="bcs")
                        nc.vector.tensor_copy(bcs[:], bc[:])
                        # write the normalized value over ALL 128 rows of the
                        # destination: pad rows get exact zeros (ps pads are
                        # zero), RoPE rows are overwritten below.
                        nc.vector.tensor_mul(dst[:, tsl], ps[:], bcs[:])
                        # partition-block swap via a PE permutation matmul
                        # (cross-partition moves aren't expressible on DVE);
                        # the swapped copy stays in PSUM and the two sin
                        # multiplies read it there against the f32 sin table.
                        qsh = mmb_pool.tile([128, THS], f32, tag="mmb",
                                            name="qsh")
                        nc.tensor.matmul(qsh[:], perm_t, dst[:, tsl],
                                         start=True, stop=True)
                        asl = slice(th * THS, (th + 1) * THS)
                        bsl = slice(T + th * THS, T + (th + 1) * THS)
                        tm1 = t1.tile([128, THS], bf16, tag="tm1")
                        tm2 = t1.tile([128, THS], bf16, tag="tm2")
                        nc.vector.tensor_mul(tm1[0:48, :], dst[0:48, tsl],
                                             tabs_t[0:48, asl])
                        nc.vector.tensor_mul(tm2[64:112, :],
                                             dst[64:112, tsl],
                                             tabs_t[64:112, bsl])
                        nc.vector.tensor_mul(tm2[0:48, :], qsh[0:48, :],
                                             tabsw_t[0:48, tsl])
                        nc.vector.tensor_mul(tm1[64:112, :], qsh[64:112, :],
                                             tabsw_t[64:112, tsl])
                        nc.vector.tensor_sub(dst[0:48, tsl], tm1[0:48, :],
                                             tm2[0:48, :])
                        nc.vector.tensor_add(dst[64:112, tsl],
                                             tm1[64:112, :], tm2[64:112, :])

                    steps.extend([part1, lambda: None, lambda: None, part2])

                def v_path(idx, th, pso):
                    # PE transposes (the XBAR dma_start_transpose SBUF->SBUF
                    # path returns garbage on real HW; CoreSim models it fine)
                    vt = t1.tile([HD, THS], f32, tag="vt")
                    nc.scalar.copy(vt[:], pso)
                    for c in range(THS // 128):
                        kidx = th * (THS // 128) + c
                        tp = mmb_pool.tile([128, THS], f32, tag="mmb",
                                           name="tp")
                        nc.tensor.transpose(tp[:, 0:HD],
                                            vt[:, c * 128:(c + 1) * 128],
                                            ident[0:HD, 0:HD])
                        nc.scalar.copy(vext[idx][:, kidx, 0:HD],
                                       tp[:, 0:HD])

                outs = [("k", 0), ("v", 0), ("k", 1), ("v", 1)] + \
                       [("q", i) for i in range(QH)]

                def w_load(kind, idx):
                    if kind == "v":
                        w_t = w_pool.tile([128, KT, HD], bf16, tag="wv",
                                          name="wv")
                        nc.sync.dma_start(w_t[:], wvt_d[idx])
                        return w_t, HD
                    w_t = w_pool.tile([128, KT, 128], bf16, tag="w", name="w")
                    nc.sync.dma_start(
                        w_t[:], (wqt_d if kind == "q" else wkt_d)[idx])
                    return w_t, 128

                # DMA emission order is the serial HWDGE/transfer order: the
                # first two weight tiles, then x token-half 0 in kt chunks,
                # then the consts, then x half 1.
                w_pre = [w_load(*outs[0]), w_load(*outs[1])]
                for th in range(TH):
                    for kc in range(4):
                        ksl = slice(kc * (KT // 4), (kc + 1) * (KT // 4))
                        tsl = slice(th * THS, (th + 1) * THS)
                        nc.sync.dma_start(xt_t[:, ksl, tsl],
                                          xt_d[:, ksl, tsl])
                    if th == 0:
                        nc.sync.dma_start(crow[:], crow_d[:])
                        nc.sync.dma_start(ones128[:], o128_d[:])
                        nc.sync.dma_start(tabs_t[:], tabs_d[:])
                        nc.sync.dma_start(tabsw_t[:], tabsw_d[:])
                        load_vext_ones()

                for oi, (kind, idx) in enumerate(outs):
                    # drip o_proj weight loads mid-stream (needed in phase 3)
                    if oi in (5, 7):
                        half = slice(0, NT // 2) if oi == 5 else \
                            slice(NT // 2, NT)
                        nc.sync.dma_start(wop_t[:, half, :],
                                          wop_d[:, half, :])
                    if oi < 2:
                        w_t, mdim = w_pre[oi]
                    else:
                        w_t, mdim = w_load(kind, idx)
                    for th in range(TH):
                        tsl = slice(th * THS, (th + 1) * THS)
                        ps = ps_pool.tile([128, THS], f32, tag="ps")
                        pso = ps[0:mdim, :]
                        for kt in range(KT):
                            nc.tensor.matmul(
                                pso, w_t[:, kt, :], xt_t[:, kt, tsl],
                                start=(kt == 0), stop=(kt == KT - 1))
                            if kt % 2 == 1 and steps:
                                steps.pop(0)()
                        if kind == "v":
                            v_path(idx, th, pso)
                        else:
                            rms_rope(kind, idx, th, ps)
                    if kind == "q":
                        steps.extend(make_attn_steps(idx))

                # drain until attn(QH-1) qc0 is normalized and scattered (its
                # atp rows gate phase-3 token-half 0); qc1 tail zips into the
                # o_proj stream below.
                while steps and not attn_done.get((QH - 1, 0)):
                    steps.pop(0)()
            # ps/w/tmp pools freed -> 2 PSUM banks for phase 3

            # ---------------- Phase 3: o_proj partial -----------------------
            ps3_pool = ctx.enter_context(
                tc.tile_pool(name="ps3", bufs=2, space="PSUM"))
            ob_pool = ctx.enter_context(tc.tile_pool(name="ob", bufs=2))
            for i in range(T // 128):
                if i == T // 128 // 2:
                    while steps:  # attn tail must land before token half 2
                        steps.pop(0)()
                isl = slice(i * 128, (i + 1) * 128)
                last_i = i == T // 128 - 1
                ob = ob_pool.tile([128, D], f32, tag="ob")
                for j in range(NJ):
                    jsl = slice(j * 512, (j + 1) * 512)
                    ps3 = ps3_pool.tile([128, 512], f32, tag="ps3")
                    for t in range(NT):
                        nc.tensor.matmul(
                            ps3[:], atp[t][:, isl], wop_t[:, t, jsl],
                            start=(t == 0), stop=(t == NT - 1))
                        if t % 2 == 1 and steps:
                            steps.pop(0)()
                    nc.scalar.copy(ob[:, jsl], ps3[:])
                    if last_i:  # per-j stores so the final drain is short
                        nc.sync.dma_start(out_d[isl, jsl], ob[:, jsl])
                if not last_i:
                    nc.sync.dma_start(out_d[isl, :], ob[:])

            if probes:
                nc.sync.dma_start(pq_d[:], qt[0][:])
                nc.sync.dma_start(pk_d[:], ktl[0][:])
                nc.sync.dma_start(pv_d[:], vext[0][:])
                nc.sync.dma_start(pa_d[:], atp[0][:])

    nc.compile()
    return nc


def get_nc():
    if "nc" not in _BUILD_CACHE:
        _BUILD_CACHE["nc"] = _build_nc()
    return _BUILD_CACHE["nc"]


def _permpad_rows(w96):
    """(96, N) head rows -> (128, N): evens at 0:48, odds at 64:112, pad 0."""
    out = np.zeros((128, w96.shape[1]), np.float32)
    out[0:48] = w96[0::2]
    out[64:112] = w96[1::2]
    return out


def _lhsT_tiles(wT, m):
    """(D, m) -> (128, KT, m) lhsT tile layout (contraction on partitions)."""
    return np.ascontiguousarray(
        wT.reshape(KT, 128, m).transpose(1, 0, 2)).astype(np.float32)


def prepare_in_maps(x, wq, wk, wv, wo, q_norm_w, k_norm_w, cos, sin):
    import ml_dtypes

    bf16 = ml_dtypes.bfloat16
    x = np.asarray(x, np.float32)
    wq = np.asarray(wq, np.float32)
    wk = np.asarray(wk, np.float32)
    wv = np.asarray(wv, np.float32)
    wo = np.asarray(wo, np.float32)
    cos = np.asarray(cos, np.float32)
    sin = np.asarray(sin, np.float32)
    q_norm_w = np.asarray(q_norm_w, np.float32)
    k_norm_w = np.asarray(k_norm_w, np.float32)

    tabs = np.zeros((128, 2 * T + KTOK + 128), np.float32)
    tabs[0:48, 0:T] = cos[:, 0::2].T
    tabs[64:112, 0:T] = sin[:, 1::2].T
    tabs[0:48, T:2 * T] = sin[:, 0::2].T
    tabs[64:112, T:2 * T] = cos[:, 1::2].T
    tabs[:, 2 * T:2 * T + KTOK] = 1.0  # vext denominator ones column
    # swap permutation (lhsT): out[i] = in[swap(i)], swap: 0:48 <-> 64:112
    for i in range(48):
        tabs[64 + i, 2 * T + KTOK + i] = 1.0
        tabs[i, 2 * T + KTOK + 64 + i] = 1.0
    # sin table in permuted layout for the PSUM-side multiplies
    tabsw = np.zeros((128, T), np.float32)
    tabsw[0:48] = sin[:, 0::2].T
    tabsw[64:112] = sin[:, 1::2].T
    crow = np.zeros((1, 352), np.float32)
    crow[0, 0:48] = q_norm_w[0::2]
    crow[0, 64:112] = q_norm_w[1::2]
    crow[0, 128:176] = k_norm_w[0::2]
    crow[0, 192:240] = k_norm_w[1::2]
    crow[0, 256:256 + HD] = 1.0

    xts = []
    for b in range(B):
        xT = np.ascontiguousarray(x[b].T)  # (D, T)
        xts.append(np.ascontiguousarray(
            xT.reshape(KT, 128, T).transpose(1, 0, 2)).astype(bf16))

    in_maps = []
    for c in range(NCORES):
        b, g = divmod(c, G)
        wqt = np.stack([
            _lhsT_tiles(_permpad_rows(
                wq[(g * QH + i) * HD:(g * QH + i + 1) * HD]).T, 128)
            for i in range(QH)]).astype(bf16)
        wkt = np.stack([
            _lhsT_tiles(_permpad_rows(
                wk[(g * KVH + i) * HD:(g * KVH + i + 1) * HD]).T, 128)
            for i in range(KVH)]).astype(bf16)
        wvt = np.stack([
            _lhsT_tiles(np.ascontiguousarray(
                wv[(g * KVH + i) * HD:(g * KVH + i + 1) * HD].T), HD)
            for i in range(KVH)]).astype(bf16)
        wo_sh = wo[:, g * QH * HD:(g + 1) * QH * HD]  # (D, 768)
        # (128, NT, D): partition p of contraction tile t is packed row
        # 128*t + p of wo_sh.T
        wop = np.ascontiguousarray(
            wo_sh.T.reshape(NT, 128, D).transpose(1, 0, 2)).astype(bf16)
        in_maps.append({
            "xt": xts[b], "wqt": wqt, "wkt": wkt, "wvt": wvt, "wop": wop,
            "tabs": tabs.astype(bf16), "tabsw": tabsw, "crow": crow,
            "o128": np.ones((128, 1), np.float32),
        })
    return in_maps


def kernel(**inputs):
    from concourse import bass_utils

    nc = get_nc()
    in_maps = prepare_in_maps(
        inputs["x"], inputs["wq"], inputs["wk"], inputs["wv"], inputs["wo"],
        inputs["q_norm_w"], inputs["k_norm_w"], inputs["cos"], inputs["sin"])
    trace = bool(int(os.environ.get("BASS_KERNEL_TRACE", "0")))
    res = bass_utils.run_bass_kernel_spmd(
        nc, in_maps, core_ids=list(range(NCORES)), trace=trace)
    _BUILD_CACHE["last_result"] = res
    partials = [np.asarray(r["out"]) for r in res.results]
    out = np.empty((B, T, D), np.float32)
    for b in range(B):
        out[b] = np.sum(np.stack(partials[b * G:(b + 1) * G]), axis=0,
                        dtype=np.float64).astype(np.float32)
    return out


# revision 60
# speedup vs baseline: 1.0307x; 1.0307x over previous
"""Grouped-Query Attention block (RMSNorm + RoPE + causal GQA + o_proj) on 8 trn2 NeuronCores.

Sharding: data-parallel over batch (2) x tensor-parallel over kv-head groups (4).
Core c = b*4 + g handles batch b, kv heads {2g, 2g+1}, q heads {8g..8g+7}.
Each core computes a partial o_proj output (T, D) over its 768 head-dims;
host sums the 4 group partials per batch.

v2 design notes (vs the fp32r baseline):
  * All matmul operands are bf16 (halves DMA + SBUF); accumulation stays fp32
    in PSUM. fp32r and bf16 both stream 1 row/cycle on TRN2, so PE time is
    unchanged but DMA drops ~2x. RMSNorm/softmax scalars stay fp32.
  * Loop order: token-halves INSIDE each projection output, so every weight
    tile is DMA'd exactly once (the old kernel fetched all weights twice).
    x stays fully resident (6.3MB bf16).
  * Attention for head h is software-pipelined INTO the projection stream of
    head h+1: one sc/pv step is emitted every 2 projection matmuls, so the
    exp (Act) chain overlaps PE work instead of gating it.
  * o_proj contraction packed 8x96 -> 6x128 (attention outputs scattered into
    packed tiles via SBUF-SBUF DMA), saving 2 accumulation matmuls per tile.
  * V transposes use the XBAR dma_start_transpose (bf16) instead of PE
    transposes + PSUM round-trip.
  * Engine balance: exp/squares/sqrt/vt/ob on Act, broadcast copies on Pool
    (gpsimd), RoPE (all-bf16, 2x DVE mode) + reciprocals + norm muls on DVE.
  * Causal mask applied structurally: above-diagonal k-tiles skipped,
    diagonal tiles masked with affine_select (fill 0 post-exp).
"""

import os
import sys

import numpy as np

sys.path.insert(0, "/opt/trn_rl_repo")

B, T, D = 2, 1024, 3072
NH, NKV, HD = 32, 8, 96
G = 4                 # tensor-parallel groups
QH = NH // G          # q heads per core (8)
KVH = NKV // G        # kv heads per core (2)
NCORES = 8
EPS = 1e-6
SCALE = 1.0 / float(np.sqrt(HD))
KT = D // 128         # 24 contraction tiles over d_model
TH = 2                # token halves per projection output
THS = T // TH         # 512
QC = 2                # q chunks in attention
QCS = T // QC         # 512
KTOK = T // 128       # 8 k tiles over tokens
NJ = D // 512         # 6 output column chunks
NT = QH * HD // 128   # 6 packed o_proj contraction tiles

_BUILD_CACHE = {}


def _build_nc():
    from contextlib import ExitStack
    from concourse import bacc, tile, mybir

    f32 = mybir.dt.float32
    f32r = mybir.dt.float32r
    bf16 = mybir.dt.bfloat16
    AF = mybir.ActivationFunctionType

    nc = bacc.Bacc("TRN2", target_bir_lowering=False, debug=False,
                   num_devices=NCORES)

    xt_d = nc.dram_tensor("xt", (128, KT, T), bf16, kind="ExternalInput").ap()
    wqt_d = nc.dram_tensor("wqt", (QH, 128, KT, 128), bf16, kind="ExternalInput").ap()
    wkt_d = nc.dram_tensor("wkt", (KVH, 128, KT, 128), bf16, kind="ExternalInput").ap()
    wvt_d = nc.dram_tensor("wvt", (KVH, 128, KT, HD), bf16, kind="ExternalInput").ap()
    wop_d = nc.dram_tensor("wop", (128, NT, D), bf16, kind="ExternalInput").ap()
    # row consts blob: [qnw(128) | knw(128) | ones96(96)] on one partition
    crow_d = nc.dram_tensor("crow", (1, 352), f32r, kind="ExternalInput").ap()
    o128_d = nc.dram_tensor("o128", (128, 1), f32r, kind="ExternalInput").ap()
    # RoPE tables blob: [taba(T) | tabb(T) | ones for vext(8) | swap-perm P]
    tabs_d = nc.dram_tensor("tabs", (128, 2 * T + KTOK + 128), bf16,
                            kind="ExternalInput").ap()
    # sin table in permuted layout (f32: multiplied against PSUM operands)
    tabsw_d = nc.dram_tensor("tabsw", (128, T), f32, kind="ExternalInput").ap()
    out_d = nc.dram_tensor("out", (T, D), f32, kind="ExternalOutput").ap()
    probes = bool(int(os.environ.get("BASS_PROBES", "0")))
    if probes:
        pq_d = nc.dram_tensor("pq", (128, T), bf16, kind="ExternalOutput").ap()
        pk_d = nc.dram_tensor("pk", (128, T), bf16, kind="ExternalOutput").ap()
        pv_d = nc.dram_tensor("pv", (128, KTOK, HD + 1), bf16,
                              kind="ExternalOutput").ap()
        pa_d = nc.dram_tensor("pa", (128, T), bf16, kind="ExternalOutput").ap()

    with tile.TileContext(nc) as tc:
        with nc.allow_low_precision(reason="bf16 matmuls, fp32 accum"), \
             ExitStack() as ctx:
            const = ctx.enter_context(tc.tile_pool(name="const", bufs=1))
            p_per = ctx.enter_context(tc.tile_pool(name="p_per", bufs=1))

            eps_t = const.tile([1, 1], f32, tag="eps")
            nc.vector.memset(eps_t[:], EPS)
            ident = const.tile([128, 128], f32, tag="ident")
            from concourse.masks import make_identity
            make_identity(nc, ident[:])
            crow = const.tile([1, 352], f32r, tag="crow")
            qnw_t = crow[:, 0:128]
            knw_t = crow[:, 128:256]
            ones96 = crow[:, 256:256 + HD]
            ones128 = const.tile([128, 1], f32r, tag="ones128")
            tabs_t = const.tile([128, 2 * T + KTOK + 128], bf16, tag="tabs")
            perm_t = tabs_t[:, 2 * T + KTOK:2 * T + KTOK + 128]
            tabsw_t = const.tile([128, T], f32, tag="tabsw")

            # x resident (both token halves), loaded once. Emission order
            # matters: the serial DMA stream must deliver the first two weight
            # tiles BEFORE x, and x token-half 0 before half 1, so the first
            # projection starts ~1.5us in instead of ~15us.
            xt_t = p_per.tile([128, KT, T], bf16, tag="xt")

            # persistent per-head tensors
            qt = [p_per.tile([128, T], bf16, tag=f"qt{h}", name=f"qt{h}")
                  for h in range(QH)]
            ktl = [p_per.tile([128, T], bf16, tag=f"kt{g2}", name=f"kt{g2}")
                   for g2 in range(KVH)]
            vext = [p_per.tile([128, KTOK, HD + 1], bf16, tag=f"vx{g2}",
                               name=f"vx{g2}") for g2 in range(KVH)]
            atp = [p_per.tile([128, T], bf16, tag=f"atp{t}", name=f"atp{t}")
                   for t in range(NT)]
            wop_t = p_per.tile([128, NT, D], bf16, tag="wop")

            # qt/ktl pad rows get exact zeros from the full-width norm
            # multiply (see part2); vext's softmax-denominator ones column is
            # DMA'd straight from the DRAM tabs blob tail (DRAM->SBUF with a
            # strided destination is the baseline-proven pattern).
            def load_vext_ones():
                for g2 in range(KVH):
                    nc.sync.dma_start(vext[g2][:, :, HD:HD + 1],
                                      tabs_d[:, 2 * T:2 * T + KTOK])

            # attention-side pools (live through phase 3 zip)
            sc_pool = ctx.enter_context(
                tc.tile_pool(name="sc", bufs=2, space="PSUM"))
            po_pool = ctx.enter_context(
                tc.tile_pool(name="po", bufs=2, space="PSUM"))
            mmb_pool = ctx.enter_context(
                tc.tile_pool(name="mmb", bufs=2, space="PSUM"))
            pt_pool = ctx.enter_context(tc.tile_pool(name="pt", bufs=3))
            at_pool = ctx.enter_context(tc.tile_pool(name="att", bufs=2))

            # ---------------- attention step machinery ----------------------
            attn_done = {}  # (head, qc) -> True once normalization emitted

            def make_attn_steps(h):
                g2 = h // (QH // KVH)
                S = [(0, k) for k in range(4)] + [(1, k) for k in range(8)]
                pts = {}
                pos = {}

                def emit_sc(j):
                    qc, kt2 = S[j]
                    qsl = slice(qc * QCS, (qc + 1) * QCS)
                    sc = sc_pool.tile([128, QCS], f32, tag="sc")
                    nc.tensor.matmul(
                        sc[:], ktl[g2][:, kt2 * 128:(kt2 + 1) * 128],
                        qt[h][:, qsl], start=True, stop=True)
                    pt = pt_pool.tile([128, QCS], bf16, tag="pt")
                    nc.scalar.activation(pt[:], sc[:], AF.Exp, scale=SCALE)
                    if kt2 >= qc * (QCS // 128):
                        nc.gpsimd.affine_select(
                            pt[:], pt[:], pattern=[[1, QCS]],
                            compare_op=mybir.AluOpType.is_ge,
                            fill=0.0,
                            base=qc * QCS - kt2 * 128,
                            channel_multiplier=-1)
                    pts[j] = pt

                def emit_fin(qc, po):
                    qsl = slice(qc * QCS, (qc + 1) * QCS)
                    rinv2 = at_pool.tile([1, QCS], f32r, tag="rinv2")
                    nc.vector.reciprocal(rinv2[:], po[HD:HD + 1, :])
                    bc2 = mmb_pool.tile([128, QCS], f32, tag="mmb")
                    nc.tensor.matmul(bc2[0:HD, :], ones96, rinv2[:],
                                     start=True, stop=True)
                    bc2s = at_pool.tile([HD, QCS], f32, tag="bc2s")
                    nc.vector.tensor_copy(bc2s[:], bc2[0:HD, :])
                    att = at_pool.tile([HD, QCS], bf16, tag="attn")
                    nc.vector.tensor_mul(att[:], po[0:HD, :], bc2s[:])
                    s = HD * h
                    t0, o0 = divmod(s, 128)
                    r1 = min(128 - o0, HD)
                    # scatter on the Act HWDGE queue: these DMAs wait on the
                    # att mul, and on the sync queue they head-of-line block
                    # the just-in-time weight prefetch
                    nc.scalar.dma_start(atp[t0][o0:o0 + r1, qsl],
                                        att[0:r1, :])
                    if r1 < HD:
                        nc.scalar.dma_start(atp[t0 + 1][0:HD - r1, qsl],
                                            att[r1:HD, :])
                    attn_done[(h, qc)] = True

                def emit_pv(j, po):
                    qc, kt2 = S[j]
                    last = 3 if qc == 0 else 7
                    nc.tensor.matmul(
                        po[:], vext[g2][:, kt2, :], pts.pop(j)[:],
                        start=(kt2 == 0), stop=(kt2 == last))
                    if kt2 == last:
                        emit_fin(qc, po)

                steps = []
                L = 2
                for j in range(L):
                    steps.append(lambda j=j: emit_sc(j))

                def pv_step(j):
                    qc, kt2 = S[j]
                    if kt2 == 0:
                        pos[qc] = po_pool.tile([HD + 1, QCS], f32, tag="po",
                                               name="po")
                    emit_pv(j, pos[qc])
                    if j + L < len(S):
                        emit_sc(j + L)

                for j in range(len(S)):
                    steps.append(lambda j=j: pv_step(j))
                return steps

            # ---------------- Phase 1+2: projections + zipped attention -----
            steps = []  # pending deferred-emission steps (rms tails + attn)
            with ExitStack() as s1:
                w_pool = s1.enter_context(tc.tile_pool(name="wst", bufs=2))
                t1 = s1.enter_context(tc.tile_pool(name="tmp1", bufs=2))
                ps_pool = s1.enter_context(
                    tc.tile_pool(name="ps1", bufs=2, space="PSUM"))

                def rms_rope(kind, idx, th, ps):
                    """Emit the square inline; defer everything with a PE
                    instruction (ssq/bc matmuls) into the step queue so the
                    Act chain never gates the PE stream."""
                    tsl = slice(th * THS, (th + 1) * THS)
                    dst = qt[idx] if kind == "q" else ktl[idx]
                    nw = qnw_t if kind == "q" else knw_t
                    sq = t1.tile([128, THS], f32r, tag="sq")
                    nc.scalar.square(sq[:], ps[:])
                    box = {}

                    def part1():
                        ssq = mmb_pool.tile([128, THS], f32, tag="mmb",
                                            name="ssq")
                        nc.tensor.matmul(ssq[0:1, :], ones128[:], sq[:],
                                         start=True, stop=True)
                        rms = t1.tile([1, THS], f32, tag="rms")
                        nc.scalar.activation(rms[:], ssq[0:1, :], AF.Sqrt,
                                             bias=eps_t[:], scale=1.0 / HD)
                        rinv = t1.tile([1, THS], f32r, tag="rinv")
                        nc.vector.reciprocal(rinv[:], rms[:])
                        box["rinv"] = rinv

                    def part2a():
                        bc = mmb_pool.tile([128, THS], f32, tag="mmb",
                                           name="bc")
                        nc.tensor.matmul(bc[:], nw, box["rinv"][:],
                                         start=True, stop=True)
                        bcs = t1.tile([128, THS], f32, tag="bcs")
                        nc.vector.tensor_copy(bcs[:], bc[:])
                        # write the normalized value over ALL 128 rows of the
                        # destination: pad rows get exact zeros (ps pads are
                        # zero), RoPE rows are overwritten below.
                        nc.vector.tensor_mul(dst[:, tsl], ps[:], bcs[:])

                    def part2b():
                        # partition-block swap via a PE permutation matmul
                        # (cross-partition moves aren't expressible on DVE);
                        # the swapped copy stays in PSUM and the two sin
                        # multiplies read it there against the f32 sin table.
                        qsh = mmb_pool.tile([128, THS], f32, tag="mmb",
                                            name="qsh")
                        nc.tensor.matmul(qsh[:], perm_t, dst[:, tsl],
                                         start=True, stop=True)
                        asl = slice(th * THS, (th + 1) * THS)
                        bsl = slice(T + th * THS, T + (th + 1) * THS)
                        tm1 = t1.tile([128, THS], bf16, tag="tm1")
                        tm2 = t1.tile([128, THS], bf16, tag="tm2")
                        nc.vector.tensor_mul(tm1[0:48, :], dst[0:48, tsl],
                                             tabs_t[0:48, asl])
                        nc.vector.tensor_mul(tm2[64:112, :],
                                             dst[64:112, tsl],
                                             tabs_t[64:112, bsl])
                        nc.vector.tensor_mul(tm2[0:48, :], qsh[0:48, :],
                                             tabsw_t[0:48, tsl])
                        nc.vector.tensor_mul(tm1[64:112, :], qsh[64:112, :],
                                             tabsw_t[64:112, tsl])
                        nc.vector.tensor_sub(dst[0:48, tsl], tm1[0:48, :],
                                             tm2[0:48, :])
                        nc.vector.tensor_add(dst[64:112, tsl],
                                             tm1[64:112, :], tm2[64:112, :])

                    nop = lambda: None  # noqa: E731
                    steps.extend([nop, part1, nop, part2a, nop, part2b])

                def v_path(idx, th, pso):
                    # PE transposes (the XBAR dma_start_transpose SBUF->SBUF
                    # path returns garbage on real HW; CoreSim models it fine)
                    vt = t1.tile([HD, THS], f32, tag="vt")
                    nc.scalar.copy(vt[:], pso)
                    for c in range(THS // 128):
                        kidx = th * (THS // 128) + c
                        tp = mmb_pool.tile([128, THS], f32, tag="mmb",
                                           name="tp")
                        nc.tensor.transpose(tp[:, 0:HD],
                                            vt[:, c * 128:(c + 1) * 128],
                                            ident[0:HD, 0:HD])
                        nc.scalar.copy(vext[idx][:, kidx, 0:HD],
                                       tp[:, 0:HD])

                outs = [("k", 0), ("v", 0), ("k", 1), ("v", 1)] + \
                       [("q", i) for i in range(QH)]

                def w_load(kind, idx):
                    if kind == "v":
                        w_t = w_pool.tile([128, KT, HD], bf16, tag="wv",
                                          name="wv")
                        nc.sync.dma_start(w_t[:], wvt_d[idx])
                        return w_t, HD
                    w_t = w_pool.tile([128, KT, 128], bf16, tag="w", name="w")
                    nc.sync.dma_start(
                        w_t[:], (wqt_d if kind == "q" else wkt_d)[idx])
                    return w_t, 128

                # DMA emission order is the serial HWDGE/transfer order: the
                # first two weight tiles, then x token-half 0 in kt chunks,
                # then the consts, then x half 1.
                w_pre = [w_load(*outs[0]), w_load(*outs[1])]
                for th in range(TH):
                    for kc in range(4):
                        ksl = slice(kc * (KT // 4), (kc + 1) * (KT // 4))
                        tsl = slice(th * THS, (th + 1) * THS)
                        nc.sync.dma_start(xt_t[:, ksl, tsl],
                                          xt_d[:, ksl, tsl])
                    if th == 0:
                        nc.sync.dma_start(crow[:], crow_d[:])
                        nc.sync.dma_start(ones128[:], o128_d[:])
                        nc.sync.dma_start(tabs_t[:], tabs_d[:])
                        nc.sync.dma_start(tabsw_t[:], tabsw_d[:])
                        load_vext_ones()

                for oi, (kind, idx) in enumerate(outs):
                    # drip o_proj weight loads mid-stream (needed in phase 3)
                    if oi in (5, 7):
                        half = slice(0, NT // 2) if oi == 5 else \
                            slice(NT // 2, NT)
                        nc.sync.dma_start(wop_t[:, half, :],
                                          wop_d[:, half, :])
                    if oi < 2:
                        w_t, mdim = w_pre[oi]
                    else:
                        w_t, mdim = w_load(kind, idx)
                    for th in range(TH):
                        tsl = slice(th * THS, (th + 1) * THS)
                        ps = ps_pool.tile([128, THS], f32, tag="ps")
                        pso = ps[0:mdim, :]
                        for kt in range(KT):
                            nc.tensor.matmul(
                                pso, w_t[:, kt, :], xt_t[:, kt, tsl],
                                start=(kt == 0), stop=(kt == KT - 1))
                            if kt % 2 == 1 and steps:
                                steps.pop(0)()
                        if kind == "v":
                            v_path(idx, th, pso)
                        else:
                            rms_rope(kind, idx, th, ps)
                    if kind == "q":
                        steps.extend(make_attn_steps(idx))

                # drain until attn(QH-1) qc0 is normalized and scattered (its
                # atp rows gate phase-3 token-half 0); qc1 tail zips into the
                # o_proj stream below.
                while steps and not attn_done.get((QH - 1, 0)):
                    steps.pop(0)()
            # ps/w/tmp pools freed -> 2 PSUM banks for phase 3

            # ---------------- Phase 3: o_proj partial -----------------------
            ps3_pool = ctx.enter_context(
                tc.tile_pool(name="ps3", bufs=2, space="PSUM"))
            ob_pool = ctx.enter_context(tc.tile_pool(name="ob", bufs=2))
            for i in range(T // 128):
                if i == T // 128 // 2:
                    while steps:  # attn tail must land before token half 2
                        steps.pop(0)()
                isl = slice(i * 128, (i + 1) * 128)
                last_i = i == T // 128 - 1
                ob = ob_pool.tile([128, D], f32, tag="ob")
                for j in range(NJ):
                    jsl = slice(j * 512, (j + 1) * 512)
                    ps3 = ps3_pool.tile([128, 512], f32, tag="ps3")
                    for t in range(NT):
                        nc.tensor.matmul(
                            ps3[:], atp[t][:, isl], wop_t[:, t, jsl],
                            start=(t == 0), stop=(t == NT - 1))
                        if t % 2 == 1 and steps:
                            steps.pop(0)()
                    nc.scalar.copy(ob[:, jsl], ps3[:])
                    if last_i:  # per-j stores so the final drain is short
                        nc.scalar.dma_start(out_d[isl, jsl], ob[:, jsl])
                if not last_i:
                    nc.scalar.dma_start(out_d[isl, :], ob[:])

            if probes:
                nc.sync.dma_start(pq_d[:], qt[0][:])
                nc.sync.dma_start(pk_d[:], ktl[0][:])
                nc.sync.dma_start(pv_d[:], vext[0][:])
                nc.sync.dma_start(pa_d[:], atp[0][:])

    nc.compile()
    return nc


def get_nc():
    if "nc" not in _BUILD_CACHE:
        _BUILD_CACHE["nc"] = _build_nc()
    return _BUILD_CACHE["nc"]


def _permpad_rows(w96):
    """(96, N) head rows -> (128, N): evens at 0:48, odds at 64:112, pad 0."""
    out = np.zeros((128, w96.shape[1]), np.float32)
    out[0:48] = w96[0::2]
    out[64:112] = w96[1::2]
    return out


def _lhsT_tiles(wT, m):
    """(D, m) -> (128, KT, m) lhsT tile layout (contraction on partitions)."""
    return np.ascontiguousarray(
        wT.reshape(KT, 128, m).transpose(1, 0, 2)).astype(np.float32)


def prepare_in_maps(x, wq, wk, wv, wo, q_norm_w, k_norm_w, cos, sin):
    import ml_dtypes

    bf16 = ml_dtypes.bfloat16
    x = np.asarray(x, np.float32)
    wq = np.asarray(wq, np.float32)
    wk = np.asarray(wk, np.float32)
    wv = np.asarray(wv, np.float32)
    wo = np.asarray(wo, np.float32)
    cos = np.asarray(cos, np.float32)
    sin = np.asarray(sin, np.float32)
    q_norm_w = np.asarray(q_norm_w, np.float32)
    k_norm_w = np.asarray(k_norm_w, np.float32)

    tabs = np.zeros((128, 2 * T + KTOK + 128), np.float32)
    tabs[0:48, 0:T] = cos[:, 0::2].T
    tabs[64:112, 0:T] = sin[:, 1::2].T
    tabs[0:48, T:2 * T] = sin[:, 0::2].T
    tabs[64:112, T:2 * T] = cos[:, 1::2].T
    tabs[:, 2 * T:2 * T + KTOK] = 1.0  # vext denominator ones column
    # swap permutation (lhsT): out[i] = in[swap(i)], swap: 0:48 <-> 64:112
    for i in range(48):
        tabs[64 + i, 2 * T + KTOK + i] = 1.0
        tabs[i, 2 * T + KTOK + 64 + i] = 1.0
    # sin table in permuted layout for the PSUM-side multiplies
    tabsw = np.zeros((128, T), np.float32)
    tabsw[0:48] = sin[:, 0::2].T
    tabsw[64:112] = sin[:, 1::2].T
    crow = np.zeros((1, 352), np.float32)
    crow[0, 0:48] = q_norm_w[0::2]
    crow[0, 64:112] = q_norm_w[1::2]
    crow[0, 128:176] = k_norm_w[0::2]
    crow[0, 192:240] = k_norm_w[1::2]
    crow[0, 256:256 + HD] = 1.0

    xts = []
    for b in range(B):
        xT = np.ascontiguousarray(x[b].T)  # (D, T)
        xts.append(np.ascontiguousarray(
            xT.reshape(KT, 128, T).transpose(1, 0, 2)).astype(bf16))

    in_maps = []
    for c in range(NCORES):
        b, g = divmod(c, G)
        wqt = np.stack([
            _lhsT_tiles(_permpad_rows(
                wq[(g * QH + i) * HD:(g * QH + i + 1) * HD]).T, 128)
            for i in range(QH)]).astype(bf16)
        wkt = np.stack([
            _lhsT_tiles(_permpad_rows(
                wk[(g * KVH + i) * HD:(g * KVH + i + 1) * HD]).T, 128)
            for i in range(KVH)]).astype(bf16)
        wvt = np.stack([
            _lhsT_tiles(np.ascontiguousarray(
                wv[(g * KVH + i) * HD:(g * KVH + i + 1) * HD].T), HD)
            for i in range(KVH)]).astype(bf16)
        wo_sh = wo[:, g * QH * HD:(g + 1) * QH * HD]  # (D, 768)
        # (128, NT, D): partition p of contraction tile t is packed row
        # 128*t + p of wo_sh.T
        wop = np.ascontiguousarray(
            wo_sh.T.reshape(NT, 128, D).transpose(1, 0, 2)).astype(bf16)
        in_maps.append({
            "xt": xts[b], "wqt": wqt, "wkt": wkt, "wvt": wvt, "wop": wop,
            "tabs": tabs.astype(bf16), "tabsw": tabsw, "crow": crow,
            "o128": np.ones((128, 1), np.float32),
        })
    return in_maps


def kernel(**inputs):
    from concourse import bass_utils

    nc = get_nc()
    in_maps = prepare_in_maps(
        inputs["x"], inputs["wq"], inputs["wk"], inputs["wv"], inputs["wo"],
        inputs["q_norm_w"], inputs["k_norm_w"], inputs["cos"], inputs["sin"])
    trace = bool(int(os.environ.get("BASS_KERNEL_TRACE", "0")))
    res = bass_utils.run_bass_kernel_spmd(
        nc, in_maps, core_ids=list(range(NCORES)), trace=trace)
    _BUILD_CACHE["last_result"] = res
    partials = [np.asarray(r["out"]) for r in res.results]
    out = np.empty((B, T, D), np.float32)
    for b in range(B):
        out[b] = np.sum(np.stack(partials[b * G:(b + 1) * G]), axis=0,
                        dtype=np.float64).astype(np.float32)
    return out
